# revision 25
# baseline (speedup 1.0000x reference)
"""Trainium2 Bass kernel: 16-head MHA forward (B=2, S=2048, D=1024, HD=64).

Sharding: 8 cores, each core owns 2 heads x both batches (head-parallel).
Per core: QKV projection for its heads (fp32r matmuls), fused transposed-score
flash-style attention fully on-chip, output projection against its 128 columns
of Wo. Host sums the 8 partial outputs and adds bo.

Emission interleaves batch-1 projection work into batch-0's attention (and
batch-0's output projection into batch-1's attention) so the PE stays busy
while the scalar engine grinds through softmax exps.

Self-contained: hardcodes shapes; only needs numpy + the concourse stack that
ships in the container image.
"""

import numpy as np

B, S, D, H, HD = 2, 2048, 1024, 16, 64
NCORES = 8
HPC = H // NCORES          # heads per core = 2
FPC = HPC * 3 * HD         # Wqkv rows per core = 384
VPC = HPC * HD             # value features per core = 128
KD = D // 128              # d-chunks = 8
ST = S // 128              # s-tiles of 128 = 16
SC = S // 512              # s-chunks of 512 = 4

_BUILT = {}


def _build(reps=1):
    if reps in _BUILT:
        return _BUILT[reps]

    import concourse.tile as tile
    import concourse.mybir as mybir
    from concourse import bacc
    from concourse.masks import make_identity

    F32 = mybir.dt.float32
    F32R = mybir.dt.float32r
    EXP = mybir.ActivationFunctionType.Exp

    nc = bacc.Bacc("TRN2", target_bir_lowering=False, debug=False, num_devices=1)

    xT = nc.dram_tensor("xT", [B, D, S], F32R, kind="ExternalInput").ap()
    wqkvT = nc.dram_tensor("wqkvT", [D, FPC], F32R, kind="ExternalInput").ap()
    bq = nc.dram_tensor("bq", [128, 3], F32, kind="ExternalInput").ap()
    woT = nc.dram_tensor("woT", [VPC, D], F32R, kind="ExternalInput").ap()
    outp = nc.dram_tensor("outp", [B, S, D], F32, kind="ExternalOutput").ap()

    with tile.TileContext(nc) as tc:
        with (
            tc.tile_pool(name="const", bufs=1) as cpool,
            tc.tile_pool(name="sb", bufs=1) as sb,
            tc.tile_pool(name="ps", bufs=1, space="PSUM") as ps,
        ):
            ident = cpool.tile([128, 128], F32, name="ident")
            make_identity(nc, ident)
            ones16 = nc.const_aps.tensor(1.0, (128, ST), F32)

            # PE warm-up during the initial DMA wait: the HAM clock gate
            # starts at half rate and releases after ~4us of sustained
            # activity, so burn idle start time on throwaway fp32 matmuls.
            warm_in = cpool.tile([128, 512], F32, name="warm_in")
            nc.vector.memset(warm_in, 0.0)
            warm_ps = ps.tile([128, 512], mybir.dt.float32, tag="aux",
                              bufs=2, name="warm_ps")
            for _w in range(4):
                nc.tensor.matmul(warm_ps, ident, warm_in,
                                 start=(_w == 0), stop=(_w == 4 - 1))

            wq_sb = cpool.tile([128, KD, FPC], F32R, name="wq_sb")
            wq_src = wqkvT.rearrange("(k p) f -> p k f", p=128)
            for k in range(KD):
                nc.sync.dma_start(out=wq_sb[:, k, :], in_=wq_src[:, k, :])
            bq_sb = cpool.tile([128, 3], F32, name="bq_sb")
            nc.sync.dma_start(out=bq_sb, in_=bq)
            wo_sb = cpool.tile([VPC, D], F32R, name="wo_sb")
            for _rep in range(reps):

                # persistent per-batch tiles
                qkv = {}     # (b, g) -> (128, S) f32r; feature groups type-major:
                             # g=0 [q_h0|q_h1], g=1 [k_h0|k_h1], g=2 [v_h0|v_h1]
                vaug = {}    # (b, h) -> (128 kj, ST, HD+1) f32r, col HD = ones
                valsT = {}   # b -> (128, S) f32r

                def phase1_chunks(b):
                    for g in range(3):
                        qkv[(b, g)] = sb.tile([128, S], F32R, tag=f"qkv{g}",
                                              bufs=2, name=f"qkv_b{b}g{g}")
                    xr = xT[b].rearrange("(k p) s -> p k s", p=128)
                    for sc in range(SC):
                        def emit(b=b, sc=sc):
                            x_t = sb.tile([128, KD, 512], F32R, tag="xt", bufs=3,
                                          name=f"xt_b{b}s{sc}")
                            for k in range(KD):
                                nc.sync.dma_start(
                                    out=x_t[:, k, :],
                                    in_=xr[:, k, sc * 512:(sc + 1) * 512])
                            # k-major: the PE consumes x chunks as they land, so
                            # the first s-chunk isn't gated on the whole 2MB.
                            # g0/g1 accumulate in the two banks of one mm slot,
                            # g2 in an aux slot.
                            qk_ab = ps.tile([128, 1024], mybir.dt.float32,
                                            tag="mm", bufs=2,
                                            name=f"qkab_b{b}s{sc}")
                            qk_c = ps.tile([128, 512], mybir.dt.float32,
                                           tag="aux", bufs=2,
                                           name=f"qkc_b{b}s{sc}")
                            for k in range(KD):
                                st_, sp_ = (k == 0), (k == KD - 1)
                                nc.tensor.matmul(
                                    qk_ab[:, 0:512], wq_sb[:, k, 0:128],
                                    x_t[:, k, :], start=st_, stop=sp_)
                                nc.tensor.matmul(
                                    qk_ab[:, 512:1024], wq_sb[:, k, 128:256],
                                    x_t[:, k, :], start=st_, stop=sp_)
                                nc.tensor.matmul(
                                    qk_c, wq_sb[:, k, 256:384],
                                    x_t[:, k, :], start=st_, stop=sp_)
                            for g, src in ((0, qk_ab[:, 0:512]),
                                           (1, qk_ab[:, 512:1024]), (2, qk_c)):
                                nc.vector.tensor_scalar_add(
                                    qkv[(b, g)][:, sc * 512:(sc + 1) * 512],
                                    src, bq_sb[:, g:g + 1])
                        yield emit

                def vtrans_chunks(b):
                    for h in range(HPC):
                        def emit(b=b, h=h):
                            va = sb.tile([128, ST, HD + 1], F32R, tag="vaug",
                                         bufs=4, name=f"vaug_b{b}h{h}")
                            vaug[(b, h)] = va
                            nc.vector.tensor_copy(va[:, :, HD], ones16)
                            vsrc = qkv[(b, 2)][h * HD:(h + 1) * HD]
                            idh = ident[h * HD:(h + 1) * HD, h * HD:(h + 1) * HD]
                            for st in range(ST):
                                pt = ps.tile([128, HD], mybir.dt.float32,
                                             tag="aux", bufs=2,
                                             name=f"pt_b{b}h{h}t{st}")
                                nc.tensor.transpose(
                                    pt,
                                    vsrc[:, st * 128:(st + 1) * 128].bitcast(F32),
                                    idh)
                                nc.vector.tensor_copy(va[:, st, 0:HD], pt)
                        yield emit

                def attn_chunks(b):
                    valsT[b] = sb.tile([128, S], F32R, tag="valsT", bufs=2,
                                       name=f"valsT_b{b}")
                    for qb in range(SC):
                        for h in range(HPC):
                            def emit(b=b, h=h, qb=qb):
                                qT = qkv[(b, 0)][h * HD:(h + 1) * HD]
                                kT = qkv[(b, 1)][h * HD:(h + 1) * HD]
                                va = vaug[(b, h)]
                                qs = qT[:, qb * 512:(qb + 1) * 512]
                                v_ps = ps.tile([HD + 1, 512], mybir.dt.float32,
                                               tag="val", bufs=2,
                                               name=f"vps_b{b}h{h}q{qb}")
                                # software-pipelined: values matmuls trail
                                # the scores/exp of the next kp so the PE
                                # never sits on an exp's latency
                                aT_prev = None
                                for kp in range(ST // 2):
                                    k0, k1 = 2 * kp, 2 * kp + 1
                                    # two kj-tiles side by side in one 2-bank psum
                                    s_ps = ps.tile([128, 1024], mybir.dt.float32,
                                                   tag="mm", bufs=2,
                                                   name=f"sps_b{b}h{h}q{qb}k{kp}")
                                    nc.tensor.matmul(
                                        s_ps[:, 0:512],
                                        kT[:, k0 * 128:(k0 + 1) * 128], qs)
                                    nc.tensor.matmul(
                                        s_ps[:, 512:1024],
                                        kT[:, k1 * 128:(k1 + 1) * 128], qs)
                                    aT = sb.tile([128, 1024], F32R, tag="aT",
                                                 bufs=6,
                                                 name=f"aT_b{b}h{h}q{qb}k{kp}")
                                    nc.scalar.activation(aT, s_ps, EXP,
                                                         scale=0.125)
                                    if aT_prev is not None:
                                        pk = kp - 1
                                        nc.tensor.matmul(
                                            v_ps, va[:, 2 * pk, :],
                                            aT_prev[:, 0:512],
                                            start=(pk == 0), stop=False)
                                        nc.tensor.matmul(
                                            v_ps, va[:, 2 * pk + 1, :],
                                            aT_prev[:, 512:1024],
                                            start=False, stop=False)
                                    aT_prev = aT
                                pk = ST // 2 - 1
                                nc.tensor.matmul(
                                    v_ps, va[:, 2 * pk, :], aT_prev[:, 0:512],
                                    start=False, stop=False)
                                nc.tensor.matmul(
                                    v_ps, va[:, 2 * pk + 1, :],
                                    aT_prev[:, 512:1024],
                                    start=False, stop=True)
                                inv = sb.tile([1, 512], F32, tag="inv", bufs=2,
                                              name=f"inv_b{b}h{h}q{qb}")
                                nc.vector.reciprocal(inv, v_ps[HD:HD + 1, :])
                                bc = sb.tile([HD, 512], F32, tag="bc", bufs=2,
                                             name=f"bc_b{b}h{h}q{qb}")
                                nc.gpsimd.partition_broadcast(bc, inv)
                                nc.vector.tensor_tensor(
                                    out=valsT[b][h * HD:(h + 1) * HD,
                                                 qb * 512:(qb + 1) * 512],
                                    in0=v_ps[0:HD, :], in1=bc,
                                    op=mybir.AluOpType.mult)
                            yield emit

                def phase3_chunks(b):
                    for st in range(ST):
                        def emit(b=b, st=st):
                            o_sb = sb.tile([128, 1024], F32, tag="osb", bufs=4,
                                           name=f"osb_b{b}t{st}")
                            for jc in range(2):
                                o_ps = ps.tile([128, 512], mybir.dt.float32,
                                               tag="aux", bufs=2,
                                               name=f"ops_b{b}t{st}j{jc}")
                                nc.tensor.matmul(
                                    o_ps,
                                    valsT[b][:, st * 128:(st + 1) * 128],
                                    wo_sb[:, jc * 512:(jc + 1) * 512])
                                nc.vector.tensor_copy(
                                    o_sb[:, jc * 512:(jc + 1) * 512], o_ps)
                            nc.sync.dma_start(
                                out=outp[b, st * 128:(st + 1) * 128, :],
                                in_=o_sb)
                        yield emit


                for c in phase1_chunks(0):
                    c()
                for c in vtrans_chunks(0):
                    c()
                # wo needed by the phase-3 fillers below; load during seg3's
                # DMA-light window
                if _rep == 0:
                    nc.sync.dma_start(out=wo_sb, in_=woT)

                # seg3: b0 attention (8 slots) || b1 projection + b0 out-proj
                # (st-tiles one qb behind the valsT columns they read)
                p1b1 = list(phase1_chunks(1))
                p3b0 = list(phase3_chunks(0))
                p3b1 = list(phase3_chunks(1))
                vt1 = list(vtrans_chunks(1))
                seg3_fill = [
                    [p1b1[0]], [p1b1[1]],
                    [p1b1[2]] + p3b0[0:2], [p1b1[3]] + p3b0[2:4],
                    p3b0[4:6] + [vt1[0]], p3b0[6:8] + [vt1[1]],
                    p3b0[8:10], p3b0[10:12],
                ]
                for i, c in enumerate(attn_chunks(0)):
                    c()
                    for f in seg3_fill[i]:
                        f()

                # seg4: b1 attention (8 slots) || leftover b0 + b1 out-proj
                seg4_fill = [
                    p3b0[12:14], p3b0[14:16],
                    p3b1[0:2], p3b1[2:4], p3b1[4:6], p3b1[6:8],
                    p3b1[8:10], p3b1[10:12],
                ]
                for i, c in enumerate(attn_chunks(1)):
                    c()
                    for f in seg4_fill[i]:
                        f()
                for f in p3b1[12:16]:
                    f()

    nc.compile()
    _BUILT[reps] = nc
    return nc


def _in_maps(x, Wqkv, bqkv, Wo):
    xT = np.ascontiguousarray(x.transpose(0, 2, 1), dtype=np.float32)
    in_maps = []
    for c in range(NCORES):
        rows = slice(c * FPC, (c + 1) * FPC)
        cols = slice(c * VPC, (c + 1) * VPC)
        # permute head-major [h0:qkv | h1:qkv] rows to type-major
        # [q_h0 q_h1 | k_h0 k_h1 | v_h0 v_h1] so q/k/v of one head share a
        # base partition on chip
        wc = Wqkv[rows].reshape(HPC, 3, HD, D).transpose(1, 0, 2, 3)
        bc = bqkv[rows].reshape(HPC, 3, HD).transpose(1, 0, 2)
        in_maps.append({
            "xT": xT,
            "wqkvT": np.ascontiguousarray(
                wc.reshape(FPC, D).T, dtype=np.float32),
            "bq": np.ascontiguousarray(
                bc.reshape(3, 128).T, dtype=np.float32),
            "woT": np.ascontiguousarray(Wo[:, cols].T, dtype=np.float32),
        })
    return in_maps


def _run_device(x, Wqkv, bqkv, Wo, trace=False):
    from concourse import bass_utils

    nc = _build()
    in_maps = _in_maps(x, Wqkv, bqkv, Wo)
    kw = {}
    if trace:
        kw = dict(trace=True, trace_cores=list(range(NCORES)),
                  stitch_traces=True)
    res = bass_utils.run_bass_kernel_spmd(
        nc, in_maps, core_ids=list(range(NCORES)), **kw)
    acc = res.results[0]["outp"].astype(np.float64)
    for c in range(1, NCORES):
        acc += res.results[c]["outp"]
    return acc, res


def _numpy_fallback(x, mask, Wqkv, bqkv, Wo, bo):
    qkv = x @ Wqkv.T + bqkv
    qkv = qkv.reshape(B, S, H, 3 * HD).transpose(0, 2, 1, 3)
    q, k, v = np.split(qkv, 3, axis=-1)
    sc = np.einsum("bhqd,bhkd->bhqk", q, k) / np.sqrt(HD).astype(np.float32)
    sc = sc + mask
    sc = sc - sc.max(axis=-1, keepdims=True)
    a = np.exp(sc)
    a /= a.sum(axis=-1, keepdims=True)
    vals = np.einsum("bhqk,bhkd->bhqd", a, v)
    vals = vals.transpose(0, 2, 1, 3).reshape(B, S, D)
    return (vals @ Wo.T + bo).astype(np.float32)


def kernel(x, mask, Wqkv, bqkv, Wo, bo):
    x = np.asarray(x, dtype=np.float32)
    mask = np.asarray(mask, dtype=np.float32)
    Wqkv = np.asarray(Wqkv, dtype=np.float32)
    bqkv = np.asarray(bqkv, dtype=np.float32)
    Wo = np.asarray(Wo, dtype=np.float32)
    bo = np.asarray(bo, dtype=np.float32)
    if mask.any():
        # device kernel folds the (all-zero) mask away; fall back if nonzero
        return _numpy_fallback(x, mask, Wqkv, bqkv, Wo, bo)
    acc, _ = _run_device(x, Wqkv, bqkv, Wo)
    return (acc + bo).astype(np.float32)



# revision 26
# speedup vs baseline: 1.4011x; 1.4011x over previous
"""Trainium2 Bass kernel: 16-head MHA forward (B=2, S=2048, D=1024, HD=64).

Sharding: 8 cores, each core owns 2 heads x both batches (head-parallel).
Per core: QKV projection for its heads (fp32r matmuls), fused transposed-score
flash-style attention fully on-chip, output projection against its 128 columns
of Wo. Host sums the 8 partial outputs and adds bo.

Emission interleaves batch-1 projection work into batch-0's attention (and
batch-0's output projection into batch-1's attention) so the PE stays busy
while the scalar engine grinds through softmax exps.

Self-contained: hardcodes shapes; only needs numpy + the concourse stack that
ships in the container image.
"""

import numpy as np

B, S, D, H, HD = 2, 2048, 1024, 16, 64
NCORES = 8
HPC = H // NCORES          # heads per core = 2
FPC = HPC * 3 * HD         # Wqkv rows per core = 384
VPC = HPC * HD             # value features per core = 128
KD = D // 128              # d-chunks = 8
ST = S // 128              # s-tiles of 128 = 16
SC = S // 512              # s-chunks of 512 = 4

_BUILT = {}


def _build(reps=1):
    if reps in _BUILT:
        return _BUILT[reps]

    import concourse.tile as tile
    import concourse.mybir as mybir
    from concourse import bacc
    from concourse.masks import make_identity

    F32 = mybir.dt.float32
    F32R = mybir.dt.float32r
    EXP = mybir.ActivationFunctionType.Exp

    nc = bacc.Bacc("TRN2", target_bir_lowering=False, debug=False, num_devices=1)

    xT = nc.dram_tensor("xT", [B, D, S], F32R, kind="ExternalInput").ap()
    wqkvT = nc.dram_tensor("wqkvT", [D, FPC], F32R, kind="ExternalInput").ap()
    bq = nc.dram_tensor("bq", [128, 3], F32, kind="ExternalInput").ap()
    woT = nc.dram_tensor("woT", [VPC, D], F32R, kind="ExternalInput").ap()
    outp = nc.dram_tensor("outp", [B, S, D], F32, kind="ExternalOutput").ap()

    with tile.TileContext(nc) as tc:
        with (
            tc.tile_pool(name="const", bufs=1) as cpool,
            tc.tile_pool(name="sb", bufs=1) as sb,
            tc.tile_pool(name="ps", bufs=1, space="PSUM") as ps,
        ):
            ident = cpool.tile([128, 128], F32, name="ident")
            make_identity(nc, ident)
            ones16 = nc.const_aps.tensor(1.0, (128, ST), F32)

            # PE warm-up during the initial DMA wait: the HAM clock gate
            # starts at half rate and releases after ~4us of sustained
            # activity, so burn idle start time on throwaway fp32 matmuls.
            warm_in = cpool.tile([128, 512], F32, name="warm_in")
            nc.vector.memset(warm_in, 0.0)
            warm_ps = ps.tile([128, 512], mybir.dt.float32, tag="aux",
                              bufs=2, name="warm_ps")
            for _w in range(4):
                nc.tensor.matmul(warm_ps, ident, warm_in,
                                 start=(_w == 0), stop=(_w == 4 - 1))

            wq_sb = cpool.tile([128, KD, FPC], F32R, name="wq_sb")
            wq_src = wqkvT.rearrange("(k p) f -> p k f", p=128)
            for k in range(KD):
                nc.sync.dma_start(out=wq_sb[:, k, :], in_=wq_src[:, k, :])
            bq_sb = cpool.tile([128, 3], F32, name="bq_sb")
            nc.sync.dma_start(out=bq_sb, in_=bq)
            wo_sb = cpool.tile([VPC, D], F32R, name="wo_sb")
            for _rep in range(reps):

                # persistent per-batch tiles
                qkv = {}     # (b, g) -> (128, S) f32r; feature groups type-major:
                             # g=0 [q_h0|q_h1], g=1 [k_h0|k_h1], g=2 [v_h0|v_h1]
                vaug = {}    # (b, h) -> (128 kj, ST, HD+1) f32r, col HD = ones
                valsT = {}   # b -> (128, S) f32r

                def phase1_chunks(b):
                    for g in range(3):
                        qkv[(b, g)] = sb.tile([128, S], F32R, tag=f"qkv{g}",
                                              bufs=2, name=f"qkv_b{b}g{g}")
                    xr = xT[b].rearrange("(k p) s -> p k s", p=128)
                    for sc in range(SC):
                        def emit(b=b, sc=sc):
                            x_t = sb.tile([128, KD, 512], F32R, tag="xt", bufs=3,
                                          name=f"xt_b{b}s{sc}")
                            for k in range(KD):
                                nc.sync.dma_start(
                                    out=x_t[:, k, :],
                                    in_=xr[:, k, sc * 512:(sc + 1) * 512])
                            # k-major: the PE consumes x chunks as they land, so
                            # the first s-chunk isn't gated on the whole 2MB.
                            # g0/g1 accumulate in the two banks of one mm slot,
                            # g2 in an aux slot.
                            qk_ab = ps.tile([128, 1024], mybir.dt.float32,
                                            tag="mm", bufs=2,
                                            name=f"qkab_b{b}s{sc}")
                            qk_c = ps.tile([128, 512], mybir.dt.float32,
                                           tag="aux", bufs=2,
                                           name=f"qkc_b{b}s{sc}")
                            for k in range(KD):
                                st_, sp_ = (k == 0), (k == KD - 1)
                                nc.tensor.matmul(
                                    qk_ab[:, 0:512], wq_sb[:, k, 0:128],
                                    x_t[:, k, :], start=st_, stop=sp_)
                                nc.tensor.matmul(
                                    qk_ab[:, 512:1024], wq_sb[:, k, 128:256],
                                    x_t[:, k, :], start=st_, stop=sp_)
                                nc.tensor.matmul(
                                    qk_c, wq_sb[:, k, 256:384],
                                    x_t[:, k, :], start=st_, stop=sp_)
                            for g, src in ((0, qk_ab[:, 0:512]),
                                           (1, qk_ab[:, 512:1024]), (2, qk_c)):
                                nc.vector.tensor_scalar_add(
                                    qkv[(b, g)][:, sc * 512:(sc + 1) * 512],
                                    src, bq_sb[:, g:g + 1])
                        yield emit

                def vtrans_chunks(b):
                    for h in range(HPC):
                        def emit(b=b, h=h):
                            va = sb.tile([128, ST, HD + 1], F32R, tag="vaug",
                                         bufs=4, name=f"vaug_b{b}h{h}")
                            vaug[(b, h)] = va
                            nc.vector.tensor_copy(va[:, :, HD], ones16)
                            vsrc = qkv[(b, 2)][h * HD:(h + 1) * HD]
                            idh = ident[h * HD:(h + 1) * HD, h * HD:(h + 1) * HD]
                            for st in range(ST):
                                pt = ps.tile([128, HD], mybir.dt.float32,
                                             tag="aux", bufs=2,
                                             name=f"pt_b{b}h{h}t{st}")
                                nc.tensor.transpose(
                                    pt,
                                    vsrc[:, st * 128:(st + 1) * 128].bitcast(F32),
                                    idh)
                                nc.vector.tensor_copy(va[:, st, 0:HD], pt)
                        yield emit

                def attn_chunks(b):
                    valsT[b] = sb.tile([128, S], F32R, tag="valsT", bufs=2,
                                       name=f"valsT_b{b}")
                    for qb in range(SC):
                        for h in range(HPC):
                            def emit(b=b, h=h, qb=qb):
                                qT = qkv[(b, 0)][h * HD:(h + 1) * HD]
                                kT = qkv[(b, 1)][h * HD:(h + 1) * HD]
                                va = vaug[(b, h)]
                                qs = qT[:, qb * 512:(qb + 1) * 512]
                                v_ps = ps.tile([HD + 1, 512], mybir.dt.float32,
                                               tag="val", bufs=2,
                                               name=f"vps_b{b}h{h}q{qb}")
                                # software-pipelined: values matmuls trail
                                # the scores/exp of the next kp so the PE
                                # never sits on an exp's latency
                                aT_prev = None
                                for kp in range(ST // 2):
                                    k0, k1 = 2 * kp, 2 * kp + 1
                                    # two kj-tiles side by side in one 2-bank psum
                                    s_ps = ps.tile([128, 1024], mybir.dt.float32,
                                                   tag="mm", bufs=2,
                                                   name=f"sps_b{b}h{h}q{qb}k{kp}")
                                    nc.tensor.matmul(
                                        s_ps[:, 0:512],
                                        kT[:, k0 * 128:(k0 + 1) * 128], qs)
                                    nc.tensor.matmul(
                                        s_ps[:, 512:1024],
                                        kT[:, k1 * 128:(k1 + 1) * 128], qs)
                                    aT = sb.tile([128, 1024], F32R, tag="aT",
                                                 bufs=6,
                                                 name=f"aT_b{b}h{h}q{qb}k{kp}")
                                    nc.scalar.activation(aT, s_ps, EXP,
                                                         scale=0.125)
                                    if aT_prev is not None:
                                        pk = kp - 1
                                        nc.tensor.matmul(
                                            v_ps, va[:, 2 * pk, :],
                                            aT_prev[:, 0:512],
                                            start=(pk == 0), stop=False)
                                        nc.tensor.matmul(
                                            v_ps, va[:, 2 * pk + 1, :],
                                            aT_prev[:, 512:1024],
                                            start=False, stop=False)
                                    aT_prev = aT
                                pk = ST // 2 - 1
                                nc.tensor.matmul(
                                    v_ps, va[:, 2 * pk, :], aT_prev[:, 0:512],
                                    start=False, stop=False)
                                nc.tensor.matmul(
                                    v_ps, va[:, 2 * pk + 1, :],
                                    aT_prev[:, 512:1024],
                                    start=False, stop=True)
                                inv = sb.tile([1, 512], F32, tag="inv", bufs=2,
                                              name=f"inv_b{b}h{h}q{qb}")
                                nc.vector.reciprocal(inv, v_ps[HD:HD + 1, :])
                                bc = sb.tile([HD, 512], F32, tag="bc", bufs=2,
                                             name=f"bc_b{b}h{h}q{qb}")
                                nc.gpsimd.partition_broadcast(bc, inv)
                                nc.vector.tensor_tensor(
                                    out=valsT[b][h * HD:(h + 1) * HD,
                                                 qb * 512:(qb + 1) * 512],
                                    in0=v_ps[0:HD, :], in1=bc,
                                    op=mybir.AluOpType.mult)
                            yield emit

                def phase3_chunks(b):
                    for st in range(ST):
                        def emit(b=b, st=st):
                            o_sb = sb.tile([128, 1024], F32, tag="osb", bufs=4,
                                           name=f"osb_b{b}t{st}")
                            for jc in range(2):
                                o_ps = ps.tile([128, 512], mybir.dt.float32,
                                               tag="aux", bufs=2,
                                               name=f"ops_b{b}t{st}j{jc}")
                                nc.tensor.matmul(
                                    o_ps,
                                    valsT[b][:, st * 128:(st + 1) * 128],
                                    wo_sb[:, jc * 512:(jc + 1) * 512])
                                nc.vector.tensor_copy(
                                    o_sb[:, jc * 512:(jc + 1) * 512], o_ps)
                                # flush each half as soon as its copy lands --
                                # the kernel-tail drain waits on the last DMA
                                nc.sync.dma_start(
                                    out=outp[b, st * 128:(st + 1) * 128,
                                             jc * 512:(jc + 1) * 512],
                                    in_=o_sb[:, jc * 512:(jc + 1) * 512])
                        yield emit


                for c in phase1_chunks(0):
                    c()
                for c in vtrans_chunks(0):
                    c()
                # wo needed by the phase-3 fillers below; load during seg3's
                # DMA-light window
                if _rep == 0:
                    nc.sync.dma_start(out=wo_sb, in_=woT)

                # seg3: b0 attention (8 slots) || b1 projection + b0 out-proj
                # (st-tiles one qb behind the valsT columns they read)
                p1b1 = list(phase1_chunks(1))
                p3b0 = list(phase3_chunks(0))
                p3b1 = list(phase3_chunks(1))
                vt1 = list(vtrans_chunks(1))
                seg3_fill = [
                    [p1b1[0]], [p1b1[1]],
                    [p1b1[2]] + p3b0[0:2], [p1b1[3]] + p3b0[2:4],
                    p3b0[4:6] + [vt1[0]], p3b0[6:8] + [vt1[1]],
                    p3b0[8:10], p3b0[10:12],
                ]
                for i, c in enumerate(attn_chunks(0)):
                    c()
                    for f in seg3_fill[i]:
                        f()

                # seg4: b1 attention (8 slots) || leftover b0 + b1 out-proj
                seg4_fill = [
                    p3b0[12:14], p3b0[14:16],
                    p3b1[0:2], p3b1[2:4], p3b1[4:6], p3b1[6:8],
                    p3b1[8:10], p3b1[10:12],
                ]
                for i, c in enumerate(attn_chunks(1)):
                    c()
                    for f in seg4_fill[i]:
                        f()
                for f in p3b1[12:16]:
                    f()

    nc.compile()
    _BUILT[reps] = nc
    return nc


def _in_maps(x, Wqkv, bqkv, Wo):
    xT = np.ascontiguousarray(x.transpose(0, 2, 1), dtype=np.float32)
    in_maps = []
    for c in range(NCORES):
        rows = slice(c * FPC, (c + 1) * FPC)
        cols = slice(c * VPC, (c + 1) * VPC)
        # permute head-major [h0:qkv | h1:qkv] rows to type-major
        # [q_h0 q_h1 | k_h0 k_h1 | v_h0 v_h1] so q/k/v of one head share a
        # base partition on chip
        wc = Wqkv[rows].reshape(HPC, 3, HD, D).transpose(1, 0, 2, 3)
        bc = bqkv[rows].reshape(HPC, 3, HD).transpose(1, 0, 2)
        in_maps.append({
            "xT": xT,
            "wqkvT": np.ascontiguousarray(
                wc.reshape(FPC, D).T, dtype=np.float32),
            "bq": np.ascontiguousarray(
                bc.reshape(3, 128).T, dtype=np.float32),
            "woT": np.ascontiguousarray(Wo[:, cols].T, dtype=np.float32),
        })
    return in_maps


def _run_device(x, Wqkv, bqkv, Wo, trace=False):
    from concourse import bass_utils

    nc = _build()
    in_maps = _in_maps(x, Wqkv, bqkv, Wo)
    kw = {}
    if trace:
        kw = dict(trace=True, trace_cores=list(range(NCORES)),
                  stitch_traces=True)
    res = bass_utils.run_bass_kernel_spmd(
        nc, in_maps, core_ids=list(range(NCORES)), **kw)
    acc = res.results[0]["outp"].astype(np.float64)
    for c in range(1, NCORES):
        acc += res.results[c]["outp"]
    return acc, res


def _numpy_fallback(x, mask, Wqkv, bqkv, Wo, bo):
    qkv = x @ Wqkv.T + bqkv
    qkv = qkv.reshape(B, S, H, 3 * HD).transpose(0, 2, 1, 3)
    q, k, v = np.split(qkv, 3, axis=-1)
    sc = np.einsum("bhqd,bhkd->bhqk", q, k) / np.sqrt(HD).astype(np.float32)
    sc = sc + mask
    sc = sc - sc.max(axis=-1, keepdims=True)
    a = np.exp(sc)
    a /= a.sum(axis=-1, keepdims=True)
    vals = np.einsum("bhqk,bhkd->bhqd", a, v)
    vals = vals.transpose(0, 2, 1, 3).reshape(B, S, D)
    return (vals @ Wo.T + bo).astype(np.float32)


def kernel(x, mask, Wqkv, bqkv, Wo, bo):
    x = np.asarray(x, dtype=np.float32)
    mask = np.asarray(mask, dtype=np.float32)
    Wqkv = np.asarray(Wqkv, dtype=np.float32)
    bqkv = np.asarray(bqkv, dtype=np.float32)
    Wo = np.asarray(Wo, dtype=np.float32)
    bo = np.asarray(bo, dtype=np.float32)
    if mask.any():
        # device kernel folds the (all-zero) mask away; fall back if nonzero
        return _numpy_fallback(x, mask, Wqkv, bqkv, Wo, bo)
    acc, _ = _run_device(x, Wqkv, bqkv, Wo)
    return (acc + bo).astype(np.float32)

